# revision 25
# baseline (speedup 1.0000x reference)
"""Trainium2 Bass kernel for nn_FFF (fast-feedforward tree routing).

Strategy (data-parallel over 8 cores, batch-sharded), all-fp16 dense
formulation with the leaf level done by gather instead of matmul:
  Per core (B_c = 8192 samples):
    1. L[b, e] = x[b] . w1[e] for the 512 ROUTING positions only
       (levels 0-8) on the PE in fp16, plus fp16 hi/lo correction terms
       for the first 128 positions (levels 0-6) so branch signs match
       fp32.  fp16 operands carry 11 mantissa bits -- same as fp32r's
       operand rounding -- so levels 7-8 raw-fp16 flips stay at the
       baseline's statistically-tolerable rate.
    2. Path mask via level-by-level recurrence on DVE (node axis on the
       free dim, level-major storage, fp16 throughout for 2x mode).
    3. C = m * L (9 nonzeros per row), Y = C @ W2 on PE (fp16), with C
       transposed 128x128 block-wise on the PE.
    4. Leaf (level 9): NO dense leaf matmul.  The chosen leaf position
       is extracted from the mask via an iota dot; ONE indirect DMA
       fetches the concatenated [w1row | w2row] (4KB fp16) for that
       leaf; lam9 = xrow . w1row on DVE (mult) + Scalar (accum);
       y += lam9 * w2row fused into the mm-B PSUM drain as a DVE
       scalar_tensor_tensor.
  Node storage permutation: level d occupies positions [2^d, 2^{d+1});
  within a level, children of level-d parents are stored [left-block |
  right-block].  Weight tables are permuted on the host to match, so
  every mask op is a contiguous slice.  Position 0 is an all-zero dummy.
  y is produced in fp16 and widened to fp32 on the host.
"""

import os
import numpy as np
from contextlib import ExitStack

import concourse.bass as bass  # noqa: F401  (AP helpers)
import concourse.tile as tile
from concourse import bacc, mybir
from concourse.bass_utils import run_bass_kernel_spmd
from concourse.masks import make_identity

F32 = mybir.dt.float32
F16 = mybir.dt.float16
I32 = mybir.dt.int32

N_CORES = 8
B_FULL, NIN, NOUT = 65536, 1024, 1024
BC = B_FULL // N_CORES          # 8192 samples per core
DEPTH = 10
NR = 512                         # routing positions (0 dummy, 1..511 lv 0-8)
NN = 1024                        # total node positions incl leaves
TB = 512                         # sample tile (4 blocks of 128)
NBLK = 128                       # PE output block (samples)

_CACHE = {}


def _build_nc():
    if "nc" in _CACHE:
        return _CACHE["nc"]
    nc = bacc.Bacc("TRN2", target_bir_lowering=False, debug=False,
                   enable_asserts=False, num_devices=N_CORES)

    n_tiles_g = BC // TB
    xt_d = nc.dram_tensor("xtp16", [n_tiles_g, 128, 8, TB], F16,
                          kind="ExternalInput").ap()
    xlo_d = nc.dram_tensor("xlop16", [n_tiles_g, 128, 8, TB], F16,
                           kind="ExternalInput").ap()
    xrow_d = nc.dram_tensor("xrow16", [BC, NIN], F16, kind="ExternalInput").ap()
    w1t_d = nc.dram_tensor("w1t16", [NIN, NR], F16, kind="ExternalInput").ap()
    w1lo_d = nc.dram_tensor("w1lo16", [NIN, 128], F16, kind="ExternalInput").ap()
    w2r_d = nc.dram_tensor("w2r16", [NR, NOUT], F16, kind="ExternalInput").ap()
    cat_d = nc.dram_tensor("cat16", [NN, 2 * NIN], F16, kind="ExternalInput").ap()
    iotab_d = nc.dram_tensor("iotab16", [128, NR], F16, kind="ExternalInput").ap()
    y_d = nc.dram_tensor("y16", [BC, NOUT], F16, kind="ExternalOutput").ap()

    with tile.TileContext(nc) as tc:
        with ExitStack() as ctx:
            statics = ctx.enter_context(tc.tile_pool(name="statics", bufs=1))
            xpool = ctx.enter_context(tc.tile_pool(name="xpool", bufs=2))
            lpool = ctx.enter_context(tc.tile_pool(name="lpool", bufs=3))
            mpool = ctx.enter_context(tc.tile_pool(name="mpool", bufs=3))
            cpool = ctx.enter_context(tc.tile_pool(name="cpool", bufs=3))
            ctpool = ctx.enter_context(tc.tile_pool(name="ctpool", bufs=4))
            gpool = ctx.enter_context(tc.tile_pool(name="gpool", bufs=3))
            ypool = ctx.enter_context(tc.tile_pool(name="ypool", bufs=3))
            psumL = ctx.enter_context(tc.tile_pool(name="psumL", bufs=3, space="PSUM"))
            psumT = ctx.enter_context(tc.tile_pool(name="psumT", bufs=2, space="PSUM"))
            psumY = ctx.enter_context(tc.tile_pool(name="psumY", bufs=3, space="PSUM"))

            ident = statics.tile([128, 128], F32)
            make_identity(nc, ident[:])
            identh = statics.tile([128, 128], F16)
            nc.vector.tensor_copy(identh[:], ident[:])

            w1t_sb = statics.tile([128, 8, NR], F16)
            nc.scalar.dma_start(w1t_sb[:], w1t_d.rearrange("(ic p) e -> p ic e", p=128))
            w1lo_sb = statics.tile([128, 8, 128], F16)
            nc.scalar.dma_start(w1lo_sb[:], w1lo_d.rearrange("(ic p) e -> p ic e", p=128))
            w2_sb = statics.tile([128, 4, NOUT], F16)
            nc.scalar.dma_start(w2_sb[:], w2r_d.rearrange("(ec p) n -> p ec n", p=128))
            iota_sb = statics.tile([128, NR], F16)
            nc.scalar.dma_start(iota_sb[:], iotab_d[:])

            n_tiles = BC // TB
            blocks_per_tile = TB // NBLK
            for t in range(n_tiles):
                xt_sb = xpool.tile([128, 8, TB], F16, tag="xt")
                nc.sync.dma_start(xt_sb[:], xt_d[t])
                xlo_sb = xpool.tile([128, 8, TB], F16, tag="xlo")
                nc.sync.dma_start(xlo_sb[:], xlo_d[t])

                L_sb = lpool.tile([128, blocks_per_tile, NR], F16)
                # ---- mm A: routing positions only, fp16 + corrections ----
                for jb in range(blocks_per_tile):
                    jsl = slice(jb * NBLK, (jb + 1) * NBLK)
                    plr = psumL.tile([128, NR], F32, tag="plr")
                    for ic in range(8):
                        nc.tensor.matmul(
                            plr[:],
                            lhsT=xt_sb[:, ic, jsl],
                            rhs=w1t_sb[:, ic, :],
                            start=(ic == 0), stop=False, skip_group_check=True,
                        )
                    for ic in range(8):
                        nc.tensor.matmul(
                            plr[:, 0:128],
                            lhsT=xlo_sb[:, ic, jsl],
                            rhs=w1t_sb[:, ic, 0:128],
                            start=False, stop=False,
                        )
                    for ic in range(8):
                        nc.tensor.matmul(
                            plr[:, 0:128],
                            lhsT=xt_sb[:, ic, jsl],
                            rhs=w1lo_sb[:, ic, :],
                            start=False, stop=(ic == 7),
                        )
                    nc.scalar.copy(L_sb[:, jb, :], plr[:])

                # ---- routing masks: [notgt|gt] table + fused recurrence ----
                gcat = mpool.tile([128, blocks_per_tile, 2, NR], F16, tag="gcat")
                nc.vector.tensor_single_scalar(
                    gcat[:, :, 0, :], L_sb[:], 0.0, mybir.AluOpType.is_le)
                nc.vector.tensor_single_scalar(
                    gcat[:, :, 1, :], L_sb[:], 0.0, mybir.AluOpType.is_gt)
                m_sb = mpool.tile([128, blocks_per_tile, NN], F16, tag="m")
                nc.vector.memset(m_sb[:, :, 0:2], 0.0)
                nc.vector.memset(m_sb[:, :, 1:2], 1.0)
                for d in range(6):   # small levels: one 2-D op for both blocks
                    sv = 2 ** d
                    n = 2 ** d
                    nc.vector.tensor_mul(
                        m_sb[:, :, 2 * sv: 2 * sv + 2 * n].rearrange(
                            "p jb (two n) -> p jb two n", two=2),
                        m_sb[:, :, sv: sv + n].unsqueeze(2).broadcast_to(
                            [128, blocks_per_tile, 2, n]),
                        gcat[:, :, :, sv: sv + n])
                for jb in range(blocks_per_tile):
                    for d in range(6, DEPTH - 1):   # big levels: 1-D per block
                        sv = 2 ** d
                        n = 2 ** d
                        nc.vector.tensor_mul(
                            m_sb[:, jb, 2 * sv: 2 * sv + 2 * n].rearrange(
                                "p (two n) -> p two n", two=2),
                            m_sb[:, jb, sv: sv + n].unsqueeze(1).broadcast_to(
                                [128, 2, n]),
                            gcat[:, jb, :, sv: sv + n])

                # ---- leaf (level 9): per-sample position from the mask ----
                scr = mpool.tile([128, NR], F16, tag="scr")
                trash = mpool.tile([128, NR], F16, tag="trash")
                trash2 = mpool.tile([128, NIN], F16, tag="trash2")
                pos9f = mpool.tile([128, blocks_per_tile, 1], F32, tag="pos9f")
                lam9 = mpool.tile([128, blocks_per_tile, 1], F32, tag="lam9")
                pos9i = mpool.tile([128, blocks_per_tile, 1], I32, tag="pos9i")
                for jb in range(blocks_per_tile):
                    nc.vector.tensor_mul(scr[:], m_sb[:, jb, NR:NN], iota_sb[:])
                    nc.scalar.activation(trash[:], scr[:],
                                         mybir.ActivationFunctionType.Copy,
                                         accum_out=pos9f[:, jb, :])
                nc.vector.tensor_copy(pos9i[:], pos9f[:])

                # ---- C = m * L (routing positions only) ----
                C_sb = cpool.tile([128, blocks_per_tile, NR], F16)
                nc.vector.tensor_mul(C_sb[:], m_sb[:, :, 0:NR], L_sb[:])

                # ---- per block: leaf gather+dot, transpose C, mm B, axpy ----
                for jb in range(blocks_per_tile):
                    rs = t * TB + jb * NBLK
                    catg = gpool.tile([128, 2 * NIN], F16, tag="catg")
                    nc.gpsimd.indirect_dma_start(
                        out=catg[:], out_offset=None, in_=cat_d[:],
                        in_offset=bass.IndirectOffsetOnAxis(
                            ap=pos9i[:, jb, :], axis=0))
                    xrow_sb = gpool.tile([128, NIN], F16, tag="xrow")
                    nc.sync.dma_start(xrow_sb[:], xrow_d[rs: rs + NBLK, :])
                    prod = gpool.tile([128, NIN], F16, tag="prod")
                    nc.vector.tensor_mul(prod[:], xrow_sb[:], catg[:, 0:NIN])
                    nc.scalar.activation(trash2[:], prod[:],
                                         mybir.ActivationFunctionType.Copy,
                                         accum_out=lam9[:, jb, :])

                    ct_sb = ctpool.tile([128, 4, 128], F16, tag="ct")
                    pt = psumT.tile([128, NR], F16)
                    for k in range(4):
                        nc.tensor.transpose(
                            pt[:, k * 128:(k + 1) * 128],
                            C_sb[:, jb, k * 128:(k + 1) * 128], identh[:])
                    nc.scalar.copy(
                        ct_sb[:].rearrange("p a b -> p (a b)"), pt[:])

                    y_sb = ypool.tile([128, NOUT], F16)
                    for nh in range(2):
                        py = psumY.tile([128, 512], F32)
                        for ec in range(4):
                            nc.tensor.matmul(
                                py[:],
                                lhsT=ct_sb[:, ec, :],
                                rhs=w2_sb[:, ec, nh * 512:(nh + 1) * 512],
                                start=(ec == 0), stop=(ec == 3),
                            )
                        nc.vector.scalar_tensor_tensor(
                            out=y_sb[:, nh * 512:(nh + 1) * 512],
                            in0=catg[:, NIN + nh * 512: NIN + (nh + 1) * 512],
                            scalar=lam9[:, jb, :], in1=py[:],
                            op0=mybir.AluOpType.mult, op1=mybir.AluOpType.add)
                    nc.sync.dma_start(y_d[rs: rs + NBLK, :], y_sb[:])

    nc.compile()
    _CACHE["nc"] = nc
    return nc


def _build_perm():
    """perm[pos-1] = original node id for storage position pos (1..1023)."""
    perm = [0]
    nodes = [0]
    for _ in range(DEPTH - 1):
        nxt = [2 * v + 1 for v in nodes] + [2 * v + 2 for v in nodes]
        perm += nxt
        nodes = nxt
    return np.array(perm, dtype=np.int64)


def kernel(x, w1s, w2s):
    nc = _build_nc()

    perm = _build_perm()
    w1p = np.ascontiguousarray(w1s[perm])          # [1023, 1024]
    w2p = np.ascontiguousarray(w2s[perm])

    w1f = np.zeros((NN, NIN), dtype=np.float32)    # [pos, i]
    w1f[1:] = w1p
    w2f = np.zeros((NN, NOUT), dtype=np.float32)
    w2f[1:] = w2p

    w1t16 = np.ascontiguousarray(w1f[0:NR].T).astype(np.float16)   # [i, pos]
    w1r = np.ascontiguousarray(w1f[0:128].T)                       # [i, 128]
    w1hi = w1r.astype(np.float16)
    w1lo16 = (w1r - w1hi.astype(np.float32)).astype(np.float16)
    w2r16 = w2f[0:NR].astype(np.float16)
    cat16 = np.concatenate([w1f, w2f], axis=1).astype(np.float16)  # [pos, 2048]
    iotab16 = np.tile(np.arange(NR, NN, dtype=np.float32), (128, 1)).astype(np.float16)

    xt = np.ascontiguousarray(x.T)                 # [1024, 65536] f32
    xt16 = xt.astype(np.float16)
    xlo16 = (xt - xt16.astype(np.float32)).astype(np.float16)
    xrow16 = x.astype(np.float16)                  # [65536, 1024]

    n_tiles = BC // TB

    def pretile(v):
        return np.ascontiguousarray(
            v.reshape(8, 128, n_tiles, TB).transpose(2, 1, 0, 3))

    in_maps = []
    for c in range(N_CORES):
        csl = slice(c * BC, (c + 1) * BC)
        in_maps.append({
            "xtp16": pretile(xt16[:, csl]),
            "xlop16": pretile(xlo16[:, csl]),
            "xrow16": np.ascontiguousarray(xrow16[csl, :]),
            "w1t16": w1t16, "w1lo16": w1lo16, "w2r16": w2r16,
            "cat16": cat16, "iotab16": iotab16,
        })

    trace = bool(int(os.environ.get("FFF_TRACE", "0")))
    res = run_bass_kernel_spmd(nc, in_maps, core_ids=list(range(N_CORES)),
                               trace=trace)
    _CACHE["last_result"] = res
    y = np.concatenate([res.results[c]["y16"] for c in range(N_CORES)], axis=0)
    return y.astype(np.float32)


# revision 28
# speedup vs baseline: 1.0110x; 1.0110x over previous
"""Trainium2 Bass kernel for nn_FFF (fast-feedforward tree routing).

Strategy (data-parallel over 8 cores, batch-sharded), all-fp16 dense
formulation with the leaf level done by gather instead of matmul:
  Per core (B_c = 8192 samples):
    1. L[b, e] = x[b] . w1[e] for the 512 ROUTING positions only
       (levels 0-8) on the PE in fp16, plus fp16 hi/lo correction terms
       for the first 128 positions (levels 0-6) so branch signs match
       fp32.  fp16 operands carry 11 mantissa bits -- same as fp32r's
       operand rounding -- so levels 7-8 raw-fp16 flips stay at the
       baseline's statistically-tolerable rate.
    2. Path mask via level-by-level recurrence on DVE (node axis on the
       free dim, level-major storage, fp16 throughout for 2x mode).
    3. C = m * L (9 nonzeros per row), Y = C @ W2 on PE (fp16), with C
       transposed 128x128 block-wise on the PE.
    4. Leaf (level 9): NO dense leaf matmul.  The chosen leaf position
       is extracted from the mask via an iota dot; ONE indirect DMA
       fetches the concatenated [w1row | w2row] (4KB fp16) for that
       leaf; lam9 = xrow . w1row on DVE (mult) + Scalar (accum);
       y += lam9 * w2row fused into the mm-B PSUM drain as a DVE
       scalar_tensor_tensor.
  Node storage permutation: level d occupies positions [2^d, 2^{d+1});
  within a level, children of level-d parents are stored [left-block |
  right-block].  Weight tables are permuted on the host to match, so
  every mask op is a contiguous slice.  Position 0 is an all-zero dummy.
  y is produced in fp16 and widened to fp32 on the host.
"""

import os
import numpy as np
from contextlib import ExitStack

import concourse.bass as bass  # noqa: F401  (AP helpers)
import concourse.tile as tile
from concourse import bacc, mybir
from concourse.bass_utils import run_bass_kernel_spmd
from concourse.masks import make_identity

F32 = mybir.dt.float32
F16 = mybir.dt.float16
I32 = mybir.dt.int32

N_CORES = 8
B_FULL, NIN, NOUT = 65536, 1024, 1024
BC = B_FULL // N_CORES          # 8192 samples per core
DEPTH = 10
NR = 512                         # routing positions (0 dummy, 1..511 lv 0-8)
NN = 1024                        # total node positions incl leaves
TB = 512                         # sample tile (4 blocks of 128)
NBLK = 128                       # PE output block (samples)

_CACHE = {}


def _build_nc():
    if "nc" in _CACHE:
        return _CACHE["nc"]
    nc = bacc.Bacc("TRN2", target_bir_lowering=False, debug=False,
                   enable_asserts=False, num_devices=N_CORES)

    n_tiles_g = BC // TB
    xt_d = nc.dram_tensor("xtp16", [n_tiles_g, 128, 8, TB], F16,
                          kind="ExternalInput").ap()
    xlo_d = nc.dram_tensor("xlop16", [n_tiles_g, 128, 8, TB], F16,
                           kind="ExternalInput").ap()
    xrow_d = nc.dram_tensor("xrow16", [BC, NIN], F16, kind="ExternalInput").ap()
    w1t_d = nc.dram_tensor("w1t16", [NIN, NR], F16, kind="ExternalInput").ap()
    w1lo_d = nc.dram_tensor("w1lo16", [NIN, 128], F16, kind="ExternalInput").ap()
    w2r_d = nc.dram_tensor("w2r16", [NR, NOUT], F16, kind="ExternalInput").ap()
    cat_d = nc.dram_tensor("cat16", [NN, 2 * NIN], F16, kind="ExternalInput").ap()
    iotab_d = nc.dram_tensor("iotab16", [128, NR], F16, kind="ExternalInput").ap()
    y_d = nc.dram_tensor("y16", [BC, NOUT], F16, kind="ExternalOutput").ap()

    with tile.TileContext(nc) as tc:
        with ExitStack() as ctx:
            statics = ctx.enter_context(tc.tile_pool(name="statics", bufs=1))
            xpool = ctx.enter_context(tc.tile_pool(name="xpool", bufs=2))
            lpool = ctx.enter_context(tc.tile_pool(name="lpool", bufs=3))
            mpool = ctx.enter_context(tc.tile_pool(name="mpool", bufs=3))
            cpool = ctx.enter_context(tc.tile_pool(name="cpool", bufs=3))
            ctpool = ctx.enter_context(tc.tile_pool(name="ctpool", bufs=4))
            gpool = ctx.enter_context(tc.tile_pool(name="gpool", bufs=3))
            ypool = ctx.enter_context(tc.tile_pool(name="ypool", bufs=3))
            psumL = ctx.enter_context(tc.tile_pool(name="psumL", bufs=2, space="PSUM"))
            psumT = ctx.enter_context(tc.tile_pool(name="psumT", bufs=2, space="PSUM"))
            psumY = ctx.enter_context(tc.tile_pool(name="psumY", bufs=2, space="PSUM"))

            ident = statics.tile([128, 128], F32)
            make_identity(nc, ident[:])
            identh = statics.tile([128, 128], F16)
            nc.vector.tensor_copy(identh[:], ident[:])

            w1t_sb = statics.tile([128, 8, NR], F16)
            nc.scalar.dma_start(w1t_sb[:], w1t_d.rearrange("(ic p) e -> p ic e", p=128))
            w1lo_sb = statics.tile([128, 8, 128], F16)
            nc.scalar.dma_start(w1lo_sb[:], w1lo_d.rearrange("(ic p) e -> p ic e", p=128))
            w2_sb = statics.tile([128, 4, NOUT], F16)
            nc.scalar.dma_start(w2_sb[:], w2r_d.rearrange("(ec p) n -> p ec n", p=128))
            iota_sb = statics.tile([128, NR], F16)
            nc.scalar.dma_start(iota_sb[:], iotab_d[:])

            n_tiles = BC // TB
            blocks_per_tile = TB // NBLK
            for t in range(n_tiles):
                xt_sb = xpool.tile([128, 8, TB], F16, tag="xt")
                nc.sync.dma_start(xt_sb[:], xt_d[t])
                xlo_sb = xpool.tile([128, 8, TB], F16, tag="xlo")
                nc.sync.dma_start(xlo_sb[:], xlo_d[t])

                # ---- mm A: routing positions only, fp16 + corrections ----
                # L is drained into per-HALF tiles so the mask chain for
                # blocks 0-1 can start while mm A still runs on blocks 2-3.
                L_h0 = lpool.tile([128, 2, NR], F16, tag="L0")
                L_h1 = lpool.tile([128, 2, NR], F16, tag="L1")
                L_hs = [L_h0, L_h1]
                for jb in range(blocks_per_tile):
                    jsl = slice(jb * NBLK, (jb + 1) * NBLK)
                    plr = psumL.tile([128, NR], F32, tag="plr")
                    for ic in range(8):
                        nc.tensor.matmul(
                            plr[:],
                            lhsT=xt_sb[:, ic, jsl],
                            rhs=w1t_sb[:, ic, :],
                            start=(ic == 0), stop=False, skip_group_check=True,
                        )
                    for ic in range(8):
                        nc.tensor.matmul(
                            plr[:, 0:128],
                            lhsT=xlo_sb[:, ic, jsl],
                            rhs=w1t_sb[:, ic, 0:128],
                            start=False, stop=False,
                        )
                    for ic in range(8):
                        nc.tensor.matmul(
                            plr[:, 0:128],
                            lhsT=xt_sb[:, ic, jsl],
                            rhs=w1lo_sb[:, ic, :],
                            start=False, stop=(ic == 7),
                        )
                    nc.scalar.copy(L_hs[jb // 2][:, jb % 2, :], plr[:])

                # ---- per half: masks, leaf position, C = m * L ----
                pos9i_hs, lam9_hs, C_hs = [], [], []
                for h in range(2):
                    L_h = L_hs[h]
                    gcat = mpool.tile([128, 2, 2, NR], F16, tag=f"gcat{h}")
                    nc.vector.tensor_single_scalar(
                        gcat[:, :, 0, :], L_h[:], 0.0, mybir.AluOpType.is_le)
                    nc.vector.tensor_single_scalar(
                        gcat[:, :, 1, :], L_h[:], 0.0, mybir.AluOpType.is_gt)
                    m_sb = mpool.tile([128, 2, NN], F16, tag=f"m{h}")
                    nc.vector.memset(m_sb[:, :, 0:2], 0.0)
                    nc.vector.memset(m_sb[:, :, 1:2], 1.0)
                    for d in range(6):   # small levels: 2-D op, both blocks
                        sv = 2 ** d
                        n = 2 ** d
                        nc.vector.tensor_mul(
                            m_sb[:, :, 2 * sv: 2 * sv + 2 * n].rearrange(
                                "p jb (two n) -> p jb two n", two=2),
                            m_sb[:, :, sv: sv + n].unsqueeze(2).broadcast_to(
                                [128, 2, 2, n]),
                            gcat[:, :, :, sv: sv + n])
                    for j in range(2):
                        for d in range(6, DEPTH - 1):   # big levels: 1-D
                            sv = 2 ** d
                            n = 2 ** d
                            nc.vector.tensor_mul(
                                m_sb[:, j, 2 * sv: 2 * sv + 2 * n].rearrange(
                                    "p (two n) -> p two n", two=2),
                                m_sb[:, j, sv: sv + n].unsqueeze(1).broadcast_to(
                                    [128, 2, n]),
                                gcat[:, j, :, sv: sv + n])
                    scr = mpool.tile([128, NR], F16, tag=f"scr{h}")
                    trash = mpool.tile([128, NR], F16, tag=f"trash{h}")
                    pos9f = mpool.tile([128, 2, 1], F32, tag=f"pos9f{h}")
                    pos9i = mpool.tile([128, 2, 1], I32, tag=f"pos9i{h}")
                    lam9 = mpool.tile([128, 2, 1], F32, tag=f"lam9{h}")
                    for j in range(2):
                        nc.vector.tensor_mul(scr[:], m_sb[:, j, NR:NN],
                                             iota_sb[:])
                        nc.scalar.activation(trash[:], scr[:],
                                             mybir.ActivationFunctionType.Copy,
                                             accum_out=pos9f[:, j, :])
                    nc.vector.tensor_copy(pos9i[:], pos9f[:])
                    C_sb = cpool.tile([128, 2, NR], F16, tag=f"C{h}")
                    nc.vector.tensor_mul(C_sb[:], m_sb[:, :, 0:NR], L_h[:])
                    pos9i_hs.append(pos9i)
                    lam9_hs.append(lam9)
                    C_hs.append(C_sb)

                # ---- per block: leaf gather+dot, transpose C, mm B, axpy ----
                trash2 = mpool.tile([128, NIN], F16, tag="trash2")
                for jb in range(blocks_per_tile):
                    h, j = jb // 2, jb % 2
                    rs = t * TB + jb * NBLK
                    catg = gpool.tile([128, 2 * NIN], F16, tag="catg")
                    nc.gpsimd.indirect_dma_start(
                        out=catg[:], out_offset=None, in_=cat_d[:],
                        in_offset=bass.IndirectOffsetOnAxis(
                            ap=pos9i_hs[h][:, j, :], axis=0))
                    xrow_sb = gpool.tile([128, NIN], F16, tag="xrow")
                    nc.sync.dma_start(xrow_sb[:], xrow_d[rs: rs + NBLK, :])
                    prod = gpool.tile([128, NIN], F16, tag="prod")
                    nc.vector.tensor_mul(prod[:], xrow_sb[:], catg[:, 0:NIN])
                    nc.scalar.activation(trash2[:], prod[:],
                                         mybir.ActivationFunctionType.Copy,
                                         accum_out=lam9_hs[h][:, j, :])

                    ct_sb = ctpool.tile([128, 4, 128], F16, tag="ct")
                    pt = psumT.tile([128, NR], F16)
                    for k in range(4):
                        nc.tensor.transpose(
                            pt[:, k * 128:(k + 1) * 128],
                            C_hs[h][:, j, k * 128:(k + 1) * 128], identh[:])
                    nc.scalar.copy(
                        ct_sb[:].rearrange("p a b -> p (a b)"), pt[:])

                    y_sb = ypool.tile([128, NOUT], F16)
                    for nh in range(2):
                        py = psumY.tile([128, 512], F32)
                        for ec in range(4):
                            nc.tensor.matmul(
                                py[:],
                                lhsT=ct_sb[:, ec, :],
                                rhs=w2_sb[:, ec, nh * 512:(nh + 1) * 512],
                                start=(ec == 0), stop=(ec == 3),
                            )
                        nc.vector.scalar_tensor_tensor(
                            out=y_sb[:, nh * 512:(nh + 1) * 512],
                            in0=catg[:, NIN + nh * 512: NIN + (nh + 1) * 512],
                            scalar=lam9_hs[h][:, j, :], in1=py[:],
                            op0=mybir.AluOpType.mult, op1=mybir.AluOpType.add)
                    nc.sync.dma_start(y_d[rs: rs + NBLK, :], y_sb[:])

    nc.compile()
    _CACHE["nc"] = nc
    return nc


def _build_perm():
    """perm[pos-1] = original node id for storage position pos (1..1023)."""
    perm = [0]
    nodes = [0]
    for _ in range(DEPTH - 1):
        nxt = [2 * v + 1 for v in nodes] + [2 * v + 2 for v in nodes]
        perm += nxt
        nodes = nxt
    return np.array(perm, dtype=np.int64)


def kernel(x, w1s, w2s):
    nc = _build_nc()

    perm = _build_perm()
    w1p = np.ascontiguousarray(w1s[perm])          # [1023, 1024]
    w2p = np.ascontiguousarray(w2s[perm])

    w1f = np.zeros((NN, NIN), dtype=np.float32)    # [pos, i]
    w1f[1:] = w1p
    w2f = np.zeros((NN, NOUT), dtype=np.float32)
    w2f[1:] = w2p

    w1t16 = np.ascontiguousarray(w1f[0:NR].T).astype(np.float16)   # [i, pos]
    w1r = np.ascontiguousarray(w1f[0:128].T)                       # [i, 128]
    w1hi = w1r.astype(np.float16)
    w1lo16 = (w1r - w1hi.astype(np.float32)).astype(np.float16)
    w2r16 = w2f[0:NR].astype(np.float16)
    cat16 = np.concatenate([w1f, w2f], axis=1).astype(np.float16)  # [pos, 2048]
    iotab16 = np.tile(np.arange(NR, NN, dtype=np.float32), (128, 1)).astype(np.float16)

    xt = np.ascontiguousarray(x.T)                 # [1024, 65536] f32
    xt16 = xt.astype(np.float16)
    xlo16 = (xt - xt16.astype(np.float32)).astype(np.float16)
    xrow16 = x.astype(np.float16)                  # [65536, 1024]

    n_tiles = BC // TB

    def pretile(v):
        return np.ascontiguousarray(
            v.reshape(8, 128, n_tiles, TB).transpose(2, 1, 0, 3))

    in_maps = []
    for c in range(N_CORES):
        csl = slice(c * BC, (c + 1) * BC)
        in_maps.append({
            "xtp16": pretile(xt16[:, csl]),
            "xlop16": pretile(xlo16[:, csl]),
            "xrow16": np.ascontiguousarray(xrow16[csl, :]),
            "w1t16": w1t16, "w1lo16": w1lo16, "w2r16": w2r16,
            "cat16": cat16, "iotab16": iotab16,
        })

    trace = bool(int(os.environ.get("FFF_TRACE", "0")))
    res = run_bass_kernel_spmd(nc, in_maps, core_ids=list(range(N_CORES)),
                               trace=trace)
    _CACHE["last_result"] = res
    y = np.concatenate([res.results[c]["y16"] for c in range(N_CORES)], axis=0)
    return y.astype(np.float32)
